# revision 22
# baseline (speedup 1.0000x reference)
"""CenterLoss kernel for 8 Trainium2 NeuronCores.

Math: with d=DECAY, e=1-d, per-class count n_c, w'(n) = 1 - e*(2-e)/n, the
reference loss decomposes exactly (see _host terms below):

  loss*B*F = sum_i w'_i ||f_i||^2 + d^2 sum_i ||c_{l_i}||^2
             - 2 d^2 sum_i f_i.c_{l_i} - e*(2-e)*Qpair

Key collapse: for singleton classes (98.4% of samples at B=16K, C=100K)
w'(1) = d^2 exactly, so grouping the first sample of every class with its
center row gives

  loss*B*F = d^2 * sum_u ||f_first_u - c_u||^2      <- the ONLY device term
           + [O(duplicates) host corrections in float64]
           - e*(2-e)*Qpair                           (host, ~B^2/2C pairs)

Sharding: labels are sorted and split into 8 contiguous chunks of 2048
samples; each core gets the compact table of the distinct center rows its
chunk references (class-dim sharding of center_feature with label
routing) plus the feature rows of the first sample of each class, laid
out slot-aligned with the table. The device streams the two aligned
[2048, 256] bf16 tensors over two HWDGE queues, subtracts on DVE, and
Square-accumulates on ACT — a pure memory-bound streaming kernel with no
gather. The host handles only rows of duplicated classes (~1.6%).
"""

import os
import sys

import numpy as np

for _p in ("/opt/trn_rl_repo",):
    if _p not in sys.path and os.path.isdir(_p):
        sys.path.insert(0, _p)

import ml_dtypes

BF16 = ml_dtypes.bfloat16
FP8 = ml_dtypes.float8_e3m4

B = 16384
F = 256
C = 100000
DECAY = 0.99
NCORES = 8

T = B // NCORES          # table slots per core (padded)
NT = T // 128            # row-major blocks of [128, 256] per core
# stream chunk sizes (elements per partition per tensor)
CHUNKS = [512, 1024, 1024, 896, 512, 128]
HOST_PAIR_LIMIT = 2_000_000  # beyond this, fall back to full host compute

_E = 1.0 - DECAY
_QCOEF = _E * (2.0 - _E)          # 0.0199
_D2 = DECAY * DECAY               # 0.9801

_nc_cache = None
_LAST_RESULT = None


def _ensure_ntff_hook():
    """bass_utils' trace path does `from antenv.axon_hooks import ...`
    unconditionally; some agent images lack that module. Register a stub
    (and wire the real ctypes NTFF hook when available) so trace=True /
    BASS_TRACE=1 degrades gracefully instead of crashing."""
    try:
        import antenv.axon_hooks  # noqa: F401
        return
    except ImportError:
        pass
    import types

    try:
        import antenv
    except ImportError:
        return
    mod = types.ModuleType("antenv.axon_hooks")
    holder = {"h": None}
    mod.set_axon_ntff_profile_hook = lambda h: holder.__setitem__("h", h)
    mod.get_axon_ntff_profile_hook = lambda: holder["h"]
    sys.modules["antenv.axon_hooks"] = mod
    antenv.axon_hooks = mod
    try:
        import importlib.util

        so = "/opt/axon/libaxon_pjrt.so"
        boot_py = "/root/.axon_site/trn_agent_boot/trn_boot.py"
        if os.path.exists(so) and os.path.exists(boot_py):
            spec = importlib.util.spec_from_file_location("_trn_boot_hookmod", boot_py)
            tb = importlib.util.module_from_spec(spec)
            spec.loader.exec_module(tb)
            h = tb._ntff_profile_via_ctypes(so)
            if h is not None:
                mod.set_axon_ntff_profile_hook(h)
    except Exception:
        pass


def _build_bass():
    import concourse.mybir as mybir
    import concourse.tile as tile
    from concourse import bacc

    f32 = mybir.dt.float32
    bf16 = mybir.dt.bfloat16
    fp8 = mybir.dt.float8e3          # e3m4: 4 mantissa bits, range ~15.5

    nc = bacc.Bacc(None)
    ident = nc.dram_tensor("ident", [128, 128], f32, kind="ExternalInput")
    trm = nc.dram_tensor("trm", [128, NT * F], fp8, kind="ExternalInput")
    frm = nc.dram_tensor("frm", [128, NT * F], fp8, kind="ExternalInput")
    # padded to 16B/partition: 4B-per-partition DMAs showed a ~6us
    # completion-semaphore lag on hardware
    out = nc.dram_tensor("out", [128, 4], f32, kind="ExternalOutput")

    with tile.TileContext(nc) as tc:
        with (
            tc.tile_pool(name="io", bufs=1) as io,
            tc.tile_pool(name="scr", bufs=4) as scr,
            tc.psum_pool(name="ps", bufs=1) as ps,
        ):
            trm_t = io.tile([128, NT * F], dtype=fp8)
            frm_t = io.tile([128, NT * F], dtype=fp8)
            # alternate chunks across the two HWDGE queues so both tensors
            # stream concurrently and chunk c of BOTH lands early;
            # graduated sizes: small first chunk starts compute early,
            # small last chunk keeps the tail short
            lo = 0
            bounds = []
            for c, ch in enumerate(CHUNKS):
                hi = lo + ch
                bounds.append((lo, hi))
                qa, qb = (nc.sync, nc.scalar) if c % 2 == 0 else (
                    nc.scalar, nc.sync)
                qa.dma_start(frm_t[:, lo:hi], frm[:, lo:hi])
                qb.dma_start(trm_t[:, lo:hi], trm[:, lo:hi])
                lo = hi
            # identity mask for the diagonal extraction: only needed after
            # the last matmul, issue it behind the input chunks
            id_t = io.tile([128, 128], dtype=f32)
            nc.scalar.dma_start(id_t[:], ident[:, :])

            # subtract on DVE (fp8 in -> bf16 out), square-and-accumulate
            # on the Tensor engine: psum[m,n] += sum_p d[p,m] d[p,n] over
            # all 32 column chunks; the diagonal sums to ||F - Tab||^2.
            psum_t = ps.tile([128, 128], dtype=f32)
            nlast = len(CHUNKS) - 1
            for c, (lo, hi) in enumerate(bounds):
                ch = hi - lo
                df = scr.tile([128, ch], dtype=bf16, tag=f"df{ch}")
                nc.vector.tensor_tensor(out=df[:], in0=frm_t[:, lo:hi],
                                        in1=trm_t[:, lo:hi],
                                        op=mybir.AluOpType.subtract)
                nmm = ch // 128
                for k in range(nmm):
                    nc.tensor.matmul(
                        out=psum_t[:],
                        lhsT=df[:, k * 128:(k + 1) * 128],
                        rhs=df[:, k * 128:(k + 1) * 128],
                        start=(c == 0 and k == 0),
                        stop=(c == nlast and k == nmm - 1),
                    )

            res = io.tile([128, 4], dtype=f32)
            nc.gpsimd.memset(res[:], 0)
            msk = scr.tile([128, 128], dtype=f32, tag="msk")
            nc.vector.tensor_tensor(out=msk[:], in0=psum_t[:], in1=id_t[:],
                                    op=mybir.AluOpType.mult)
            nc.vector.tensor_reduce(
                out=res[:, 0:1],
                in_=msk[:].rearrange("p (n d) -> p n d", d=128),
                axis=mybir.AxisListType.X, op=mybir.AluOpType.add)

            nc.sync.dma_start(out[:, :], res[:])
    nc.finalize()
    return nc


def _get_nc():
    global _nc_cache
    if _nc_cache is None:
        _nc_cache = _build_bass()
    return _nc_cache


def _host_reference(f, labels, cf):
    """Full-precision host fallback (pathological label distributions only)."""
    f64 = f.astype(np.float64)
    sums = np.zeros((C, F), np.float64)
    np.add.at(sums, labels, f64)
    counts = np.bincount(labels, minlength=C).astype(np.float64)
    mean = sums / np.maximum(counts, 1.0)[:, None]
    newc = np.where((counts > 0)[:, None],
                    DECAY * cf.astype(np.float64) + (1 - DECAY) * mean,
                    cf.astype(np.float64))
    g = newc[labels]
    return np.float32(np.mean((f64 - g) ** 2))


def kernel(batch_feature, batch_label, center_feature):
    global _LAST_RESULT
    f = np.ascontiguousarray(np.asarray(batch_feature, dtype=np.float32))
    labels = np.asarray(batch_label).astype(np.int64)
    cf = np.ascontiguousarray(np.asarray(center_feature, dtype=np.float32))

    order = np.argsort(labels, kind="stable")
    sl = labels[order]                       # sorted labels
    uniq_all, run_start, run_cnt = np.unique(sl, return_index=True,
                                             return_counts=True)

    n_pairs_total = int(((run_cnt * (run_cnt - 1)) // 2).sum())
    if n_pairs_total > HOST_PAIR_LIMIT:
        return _host_reference(f, labels, cf)

    in_maps = []
    ident = np.eye(128, dtype=np.float32)
    host_corr = 0.0                          # O(duplicates) terms, float64
    f64 = f.astype(np.float64)
    cf64 = cf.astype(np.float64)
    for k in range(NCORES):
        seg = slice(k * T, (k + 1) * T)
        rows = order[seg]
        sl_k = sl[seg]
        uniq, first_idx, cnt = np.unique(sl_k, return_index=True,
                                         return_counts=True)
        U_k = uniq.shape[0]

        tab_k = np.zeros((T, F), np.float32)
        tab_k[:U_k] = cf[uniq]
        f_k = np.zeros((T, F), np.float32)
        f_k[:U_k] = f[rows[first_idx]]

        in_maps.append({
            "ident": ident,
            "trm": np.ascontiguousarray(tab_k.reshape(128, NT * F)).astype(FP8),
            "frm": np.ascontiguousarray(f_k.reshape(128, NT * F)).astype(FP8),
        })

        dupm = cnt >= 2
        if dupm.any():
            nd = cnt[dupm].astype(np.float64)
            wq_d = 1.0 - _QCOEF / nd
            cd = cf64[uniq[dupm]]
            fd = f64[rows[first_idx[dupm]]]
            # (A) first-sample norm weight correction (w' - d^2)
            host_corr += float((wq_d - _D2) @ (fd * fd).sum(1))
            # (C) extras' center norms: d^2 (n-1) ||c||^2
            host_corr += _D2 * float((nd - 1.0) @ (cd * cd).sum(1))
            # extras: non-first samples of duplicated classes
            is_first = np.zeros(T, bool)
            is_first[first_idx] = True
            ex = rows[~is_first]
            ex_lab = labels[ex]
            fe = f64[ex]
            ce = cf64[ex_lab]
            wq_e = 1.0 - _QCOEF / cnt[np.searchsorted(uniq, ex_lab)]
            # (B) extras' feature norms, (D) extras' cross terms
            host_corr += float(wq_e @ (fe * fe).sum(1))
            host_corr -= 2.0 * _D2 * float((fe * ce).sum())

    _ensure_ntff_hook()
    from concourse.bass_utils import run_bass_kernel_spmd

    nc = _get_nc()
    res = run_bass_kernel_spmd(nc, in_maps, core_ids=list(range(NCORES)))
    _LAST_RESULT = res

    d_total = 0.0
    for r in res.results:
        d_total += float(np.asarray(r["out"], np.float64).sum())

    # same-class pair term, float64 on host (~B^2/2C pairs)
    q2 = 0.0
    dup = np.nonzero(run_cnt >= 2)[0]
    if dup.size:
        ia_l, jb_l, wt_l = [], [], []
        for r_i in dup:
            s0, n = int(run_start[r_i]), int(run_cnt[r_i])
            g = order[s0:s0 + n]
            iu, ju = np.triu_indices(n, k=1)
            ia_l.append(g[iu]); jb_l.append(g[ju])
            wt_l.append(np.full(iu.shape[0], 2.0 / n))
        ia = np.concatenate(ia_l); jb = np.concatenate(jb_l)
        wt = np.concatenate(wt_l)
        dots = np.einsum("ij,ij->i", f64[ia], f64[jb])
        q2 = float(wt @ dots)

    loss = (_D2 * d_total + host_corr - _QCOEF * q2) / (B * F)
    return np.float32(loss)


# revision 23
# speedup vs baseline: 1.0046x; 1.0046x over previous
"""CenterLoss kernel for 8 Trainium2 NeuronCores.

Math: with d=DECAY, e=1-d, per-class count n_c, w'(n) = 1 - e*(2-e)/n, the
reference loss decomposes exactly (see _host terms below):

  loss*B*F = sum_i w'_i ||f_i||^2 + d^2 sum_i ||c_{l_i}||^2
             - 2 d^2 sum_i f_i.c_{l_i} - e*(2-e)*Qpair

Key collapse: for singleton classes (98.4% of samples at B=16K, C=100K)
w'(1) = d^2 exactly, so grouping the first sample of every class with its
center row gives

  loss*B*F = d^2 * sum_u ||f_first_u - c_u||^2      <- the ONLY device term
           + [O(duplicates) host corrections in float64]
           - e*(2-e)*Qpair                           (host, ~B^2/2C pairs)

Sharding: labels are sorted and split into 8 contiguous chunks of 2048
samples; each core gets the compact table of the distinct center rows its
chunk references (class-dim sharding of center_feature with label
routing) plus the feature rows of the first sample of each class, laid
out slot-aligned with the table. The device streams the two aligned
[2048, 256] bf16 tensors over two HWDGE queues, subtracts on DVE, and
Square-accumulates on ACT — a pure memory-bound streaming kernel with no
gather. The host handles only rows of duplicated classes (~1.6%).
"""

import os
import sys

import numpy as np

for _p in ("/opt/trn_rl_repo",):
    if _p not in sys.path and os.path.isdir(_p):
        sys.path.insert(0, _p)

import ml_dtypes

BF16 = ml_dtypes.bfloat16
FP8 = ml_dtypes.float8_e3m4

B = 16384
F = 256
C = 100000
DECAY = 0.99
NCORES = 8

T = B // NCORES          # table slots per core (padded)
NT = T // 128            # row-major blocks of [128, 256] per core
# stream chunk sizes (elements per partition per tensor)
CHUNKS = [512, 1280, 1280, 768, 256]
HOST_PAIR_LIMIT = 2_000_000  # beyond this, fall back to full host compute

_E = 1.0 - DECAY
_QCOEF = _E * (2.0 - _E)          # 0.0199
_D2 = DECAY * DECAY               # 0.9801

_nc_cache = None
_LAST_RESULT = None


def _ensure_ntff_hook():
    """bass_utils' trace path does `from antenv.axon_hooks import ...`
    unconditionally; some agent images lack that module. Register a stub
    (and wire the real ctypes NTFF hook when available) so trace=True /
    BASS_TRACE=1 degrades gracefully instead of crashing."""
    try:
        import antenv.axon_hooks  # noqa: F401
        return
    except ImportError:
        pass
    import types

    try:
        import antenv
    except ImportError:
        return
    mod = types.ModuleType("antenv.axon_hooks")
    holder = {"h": None}
    mod.set_axon_ntff_profile_hook = lambda h: holder.__setitem__("h", h)
    mod.get_axon_ntff_profile_hook = lambda: holder["h"]
    sys.modules["antenv.axon_hooks"] = mod
    antenv.axon_hooks = mod
    try:
        import importlib.util

        so = "/opt/axon/libaxon_pjrt.so"
        boot_py = "/root/.axon_site/trn_agent_boot/trn_boot.py"
        if os.path.exists(so) and os.path.exists(boot_py):
            spec = importlib.util.spec_from_file_location("_trn_boot_hookmod", boot_py)
            tb = importlib.util.module_from_spec(spec)
            spec.loader.exec_module(tb)
            h = tb._ntff_profile_via_ctypes(so)
            if h is not None:
                mod.set_axon_ntff_profile_hook(h)
    except Exception:
        pass


def _build_bass():
    import concourse.mybir as mybir
    import concourse.tile as tile
    from concourse import bacc

    f32 = mybir.dt.float32
    bf16 = mybir.dt.bfloat16
    fp8 = mybir.dt.float8e3          # e3m4: 4 mantissa bits, range ~15.5

    nc = bacc.Bacc(None)
    ident = nc.dram_tensor("ident", [128, 128], f32, kind="ExternalInput")
    trm = nc.dram_tensor("trm", [128, NT * F], fp8, kind="ExternalInput")
    frm = nc.dram_tensor("frm", [128, NT * F], fp8, kind="ExternalInput")
    # padded to 16B/partition: 4B-per-partition DMAs showed a ~6us
    # completion-semaphore lag on hardware
    out = nc.dram_tensor("out", [128, 4], f32, kind="ExternalOutput")

    with tile.TileContext(nc) as tc:
        with (
            tc.tile_pool(name="io", bufs=1) as io,
            tc.tile_pool(name="scr", bufs=4) as scr,
            tc.psum_pool(name="ps", bufs=1) as ps,
        ):
            trm_t = io.tile([128, NT * F], dtype=fp8)
            frm_t = io.tile([128, NT * F], dtype=fp8)
            # alternate chunks across the two HWDGE queues so both tensors
            # stream concurrently and chunk c of BOTH lands early;
            # graduated sizes: small first chunk starts compute early,
            # small last chunk keeps the tail short
            lo = 0
            bounds = []
            for c, ch in enumerate(CHUNKS):
                hi = lo + ch
                bounds.append((lo, hi))
                qa, qb = (nc.sync, nc.scalar) if c % 2 == 0 else (
                    nc.scalar, nc.sync)
                qa.dma_start(frm_t[:, lo:hi], frm[:, lo:hi])
                qb.dma_start(trm_t[:, lo:hi], trm[:, lo:hi])
                lo = hi
            # identity mask for the diagonal extraction: only needed after
            # the last matmul, issue it behind the input chunks
            id_t = io.tile([128, 128], dtype=f32)
            nc.scalar.dma_start(id_t[:], ident[:, :])

            # subtract on DVE (fp8 in -> bf16 out), square-and-accumulate
            # on the Tensor engine: psum[m,n] += sum_p d[p,m] d[p,n] over
            # all 32 column chunks; the diagonal sums to ||F - Tab||^2.
            psum_t = ps.tile([128, 128], dtype=f32)
            nlast = len(CHUNKS) - 1
            for c, (lo, hi) in enumerate(bounds):
                ch = hi - lo
                df = scr.tile([128, ch], dtype=bf16, tag=f"df{ch}")
                nc.vector.tensor_tensor(out=df[:], in0=frm_t[:, lo:hi],
                                        in1=trm_t[:, lo:hi],
                                        op=mybir.AluOpType.subtract)
                nmm = ch // 128
                for k in range(nmm):
                    nc.tensor.matmul(
                        out=psum_t[:],
                        lhsT=df[:, k * 128:(k + 1) * 128],
                        rhs=df[:, k * 128:(k + 1) * 128],
                        start=(c == 0 and k == 0),
                        stop=(c == nlast and k == nmm - 1),
                    )

            res = io.tile([128, 4], dtype=f32)
            nc.gpsimd.memset(res[:], 0)
            msk = scr.tile([128, 128], dtype=f32, tag="msk")
            nc.vector.tensor_tensor(out=msk[:], in0=psum_t[:], in1=id_t[:],
                                    op=mybir.AluOpType.mult)
            nc.vector.tensor_reduce(
                out=res[:, 0:1],
                in_=msk[:].rearrange("p (n d) -> p n d", d=128),
                axis=mybir.AxisListType.X, op=mybir.AluOpType.add)

            nc.sync.dma_start(out[:, :], res[:])
    nc.finalize()
    return nc


def _get_nc():
    global _nc_cache
    if _nc_cache is None:
        _nc_cache = _build_bass()
    return _nc_cache


def _host_reference(f, labels, cf):
    """Full-precision host fallback (pathological label distributions only)."""
    f64 = f.astype(np.float64)
    sums = np.zeros((C, F), np.float64)
    np.add.at(sums, labels, f64)
    counts = np.bincount(labels, minlength=C).astype(np.float64)
    mean = sums / np.maximum(counts, 1.0)[:, None]
    newc = np.where((counts > 0)[:, None],
                    DECAY * cf.astype(np.float64) + (1 - DECAY) * mean,
                    cf.astype(np.float64))
    g = newc[labels]
    return np.float32(np.mean((f64 - g) ** 2))


def kernel(batch_feature, batch_label, center_feature):
    global _LAST_RESULT
    f = np.ascontiguousarray(np.asarray(batch_feature, dtype=np.float32))
    labels = np.asarray(batch_label).astype(np.int64)
    cf = np.ascontiguousarray(np.asarray(center_feature, dtype=np.float32))

    order = np.argsort(labels, kind="stable")
    sl = labels[order]                       # sorted labels
    uniq_all, run_start, run_cnt = np.unique(sl, return_index=True,
                                             return_counts=True)

    n_pairs_total = int(((run_cnt * (run_cnt - 1)) // 2).sum())
    if n_pairs_total > HOST_PAIR_LIMIT:
        return _host_reference(f, labels, cf)

    in_maps = []
    ident = np.eye(128, dtype=np.float32)
    host_corr = 0.0                          # O(duplicates) terms, float64
    f64 = f.astype(np.float64)
    cf64 = cf.astype(np.float64)
    for k in range(NCORES):
        seg = slice(k * T, (k + 1) * T)
        rows = order[seg]
        sl_k = sl[seg]
        uniq, first_idx, cnt = np.unique(sl_k, return_index=True,
                                         return_counts=True)
        U_k = uniq.shape[0]

        tab_k = np.zeros((T, F), np.float32)
        tab_k[:U_k] = cf[uniq]
        f_k = np.zeros((T, F), np.float32)
        f_k[:U_k] = f[rows[first_idx]]

        in_maps.append({
            "ident": ident,
            "trm": np.ascontiguousarray(tab_k.reshape(128, NT * F)).astype(FP8),
            "frm": np.ascontiguousarray(f_k.reshape(128, NT * F)).astype(FP8),
        })

        dupm = cnt >= 2
        if dupm.any():
            nd = cnt[dupm].astype(np.float64)
            wq_d = 1.0 - _QCOEF / nd
            cd = cf64[uniq[dupm]]
            fd = f64[rows[first_idx[dupm]]]
            # (A) first-sample norm weight correction (w' - d^2)
            host_corr += float((wq_d - _D2) @ (fd * fd).sum(1))
            # (C) extras' center norms: d^2 (n-1) ||c||^2
            host_corr += _D2 * float((nd - 1.0) @ (cd * cd).sum(1))
            # extras: non-first samples of duplicated classes
            is_first = np.zeros(T, bool)
            is_first[first_idx] = True
            ex = rows[~is_first]
            ex_lab = labels[ex]
            fe = f64[ex]
            ce = cf64[ex_lab]
            wq_e = 1.0 - _QCOEF / cnt[np.searchsorted(uniq, ex_lab)]
            # (B) extras' feature norms, (D) extras' cross terms
            host_corr += float(wq_e @ (fe * fe).sum(1))
            host_corr -= 2.0 * _D2 * float((fe * ce).sum())

    _ensure_ntff_hook()
    from concourse.bass_utils import run_bass_kernel_spmd

    nc = _get_nc()
    res = run_bass_kernel_spmd(nc, in_maps, core_ids=list(range(NCORES)))
    _LAST_RESULT = res

    d_total = 0.0
    for r in res.results:
        d_total += float(np.asarray(r["out"], np.float64).sum())

    # same-class pair term, float64 on host (~B^2/2C pairs)
    q2 = 0.0
    dup = np.nonzero(run_cnt >= 2)[0]
    if dup.size:
        ia_l, jb_l, wt_l = [], [], []
        for r_i in dup:
            s0, n = int(run_start[r_i]), int(run_cnt[r_i])
            g = order[s0:s0 + n]
            iu, ju = np.triu_indices(n, k=1)
            ia_l.append(g[iu]); jb_l.append(g[ju])
            wt_l.append(np.full(iu.shape[0], 2.0 / n))
        ia = np.concatenate(ia_l); jb = np.concatenate(jb_l)
        wt = np.concatenate(wt_l)
        dots = np.einsum("ij,ij->i", f64[ia], f64[jb])
        q2 = float(wt @ dots)

    loss = (_D2 * d_total + host_corr - _QCOEF * q2) / (B * F)
    return np.float32(loss)


# revision 24
# speedup vs baseline: 1.0345x; 1.0297x over previous
"""CenterLoss kernel for 8 Trainium2 NeuronCores.

Math: with d=DECAY, e=1-d, per-class count n_c, w'(n) = 1 - e*(2-e)/n, the
reference loss decomposes exactly (see _host terms below):

  loss*B*F = sum_i w'_i ||f_i||^2 + d^2 sum_i ||c_{l_i}||^2
             - 2 d^2 sum_i f_i.c_{l_i} - e*(2-e)*Qpair

Key collapse: for singleton classes (98.4% of samples at B=16K, C=100K)
w'(1) = d^2 exactly, so grouping the first sample of every class with its
center row gives

  loss*B*F = d^2 * sum_u ||f_first_u - c_u||^2      <- the ONLY device term
           + [O(duplicates) host corrections in float64]
           - e*(2-e)*Qpair                           (host, ~B^2/2C pairs)

Sharding: labels are sorted and split into 8 contiguous chunks of 2048
samples; each core gets the compact table of the distinct center rows its
chunk references (class-dim sharding of center_feature with label
routing) plus the feature rows of the first sample of each class, laid
out slot-aligned with the table. The device streams the two aligned
[2048, 256] bf16 tensors over two HWDGE queues, subtracts on DVE, and
Square-accumulates on ACT — a pure memory-bound streaming kernel with no
gather. The host handles only rows of duplicated classes (~1.6%).
"""

import os
import sys

import numpy as np

for _p in ("/opt/trn_rl_repo",):
    if _p not in sys.path and os.path.isdir(_p):
        sys.path.insert(0, _p)

import ml_dtypes

BF16 = ml_dtypes.bfloat16
FP8 = ml_dtypes.float8_e3m4

B = 16384
F = 256
C = 100000
DECAY = 0.99
NCORES = 8

T = B // NCORES          # table slots per core (padded)
NT = T // 128            # row-major blocks of [128, 256] per core
# stream chunk sizes (elements per partition per tensor)
CHUNKS = [512, 1024, 1024, 896, 512, 128]
HOST_PAIR_LIMIT = 2_000_000  # beyond this, fall back to full host compute

_E = 1.0 - DECAY
_QCOEF = _E * (2.0 - _E)          # 0.0199
_D2 = DECAY * DECAY               # 0.9801

_nc_cache = None
_LAST_RESULT = None


def _ensure_ntff_hook():
    """bass_utils' trace path does `from antenv.axon_hooks import ...`
    unconditionally; some agent images lack that module. Register a stub
    (and wire the real ctypes NTFF hook when available) so trace=True /
    BASS_TRACE=1 degrades gracefully instead of crashing."""
    try:
        import antenv.axon_hooks  # noqa: F401
        return
    except ImportError:
        pass
    import types

    try:
        import antenv
    except ImportError:
        return
    mod = types.ModuleType("antenv.axon_hooks")
    holder = {"h": None}
    mod.set_axon_ntff_profile_hook = lambda h: holder.__setitem__("h", h)
    mod.get_axon_ntff_profile_hook = lambda: holder["h"]
    sys.modules["antenv.axon_hooks"] = mod
    antenv.axon_hooks = mod
    try:
        import importlib.util

        so = "/opt/axon/libaxon_pjrt.so"
        boot_py = "/root/.axon_site/trn_agent_boot/trn_boot.py"
        if os.path.exists(so) and os.path.exists(boot_py):
            spec = importlib.util.spec_from_file_location("_trn_boot_hookmod", boot_py)
            tb = importlib.util.module_from_spec(spec)
            spec.loader.exec_module(tb)
            h = tb._ntff_profile_via_ctypes(so)
            if h is not None:
                mod.set_axon_ntff_profile_hook(h)
    except Exception:
        pass


def _build_bass():
    import concourse.mybir as mybir
    import concourse.tile as tile
    from concourse import bacc

    f32 = mybir.dt.float32
    bf16 = mybir.dt.bfloat16
    fp8 = mybir.dt.float8e3          # e3m4: 4 mantissa bits, range ~15.5

    nc = bacc.Bacc(None)
    ident = nc.dram_tensor("ident", [128, 128], f32, kind="ExternalInput")
    trm = nc.dram_tensor("trm", [128, NT * F], fp8, kind="ExternalInput")
    frm = nc.dram_tensor("frm", [128, NT * F], fp8, kind="ExternalInput")
    # padded to 16B/partition: 4B-per-partition DMAs showed a ~6us
    # completion-semaphore lag on hardware
    out = nc.dram_tensor("out", [128, 4], f32, kind="ExternalOutput")

    with tile.TileContext(nc) as tc:
        with (
            tc.tile_pool(name="io", bufs=1) as io,
            tc.tile_pool(name="scr", bufs=4) as scr,
            tc.psum_pool(name="ps", bufs=1) as ps,
        ):
            trm_t = io.tile([128, NT * F], dtype=fp8)
            frm_t = io.tile([128, NT * F], dtype=fp8)
            # alternate chunks across the two HWDGE queues so both tensors
            # stream concurrently and chunk c of BOTH lands early;
            # graduated sizes: small first chunk starts compute early,
            # small last chunk keeps the tail short
            lo = 0
            bounds = []
            for c, ch in enumerate(CHUNKS):
                hi = lo + ch
                bounds.append((lo, hi))
                qa, qb = (nc.sync, nc.scalar) if c % 2 == 0 else (
                    nc.scalar, nc.sync)
                qa.dma_start(frm_t[:, lo:hi], frm[:, lo:hi])
                qb.dma_start(trm_t[:, lo:hi], trm[:, lo:hi])
                lo = hi
            # identity mask for the diagonal extraction: only needed after
            # the last matmul, issue it behind the input chunks
            id_t = io.tile([128, 128], dtype=f32)
            nc.scalar.dma_start(id_t[:], ident[:, :])

            # subtract on DVE (fp8 in -> bf16 out), square-and-accumulate
            # on the Tensor engine: psum[m,n] += sum_p d[p,m] d[p,n] over
            # all 32 column chunks; the diagonal sums to ||F - Tab||^2.
            psum_t = ps.tile([128, 128], dtype=f32)
            nlast = len(CHUNKS) - 1
            for c, (lo, hi) in enumerate(bounds):
                ch = hi - lo
                df = scr.tile([128, ch], dtype=bf16, tag=f"df{ch}")
                nc.vector.tensor_tensor(out=df[:], in0=frm_t[:, lo:hi],
                                        in1=trm_t[:, lo:hi],
                                        op=mybir.AluOpType.subtract)
                nmm = ch // 128
                for k in range(nmm):
                    nc.tensor.matmul(
                        out=psum_t[:],
                        lhsT=df[:, k * 128:(k + 1) * 128],
                        rhs=df[:, k * 128:(k + 1) * 128],
                        start=(c == 0 and k == 0),
                        stop=(c == nlast and k == nmm - 1),
                    )

            res = io.tile([128, 4], dtype=f32)
            nc.gpsimd.memset(res[:], 0)
            msk = scr.tile([128, 128], dtype=f32, tag="msk")
            nc.vector.tensor_tensor(out=msk[:], in0=psum_t[:], in1=id_t[:],
                                    op=mybir.AluOpType.mult)
            nc.vector.tensor_reduce(
                out=res[:, 0:1],
                in_=msk[:].rearrange("p (n d) -> p n d", d=128),
                axis=mybir.AxisListType.X, op=mybir.AluOpType.add)

            nc.sync.dma_start(out[:, :], res[:])
    nc.finalize()
    return nc


def _get_nc():
    global _nc_cache
    if _nc_cache is None:
        _nc_cache = _build_bass()
    return _nc_cache


def _host_reference(f, labels, cf):
    """Full-precision host fallback (pathological label distributions only)."""
    f64 = f.astype(np.float64)
    sums = np.zeros((C, F), np.float64)
    np.add.at(sums, labels, f64)
    counts = np.bincount(labels, minlength=C).astype(np.float64)
    mean = sums / np.maximum(counts, 1.0)[:, None]
    newc = np.where((counts > 0)[:, None],
                    DECAY * cf.astype(np.float64) + (1 - DECAY) * mean,
                    cf.astype(np.float64))
    g = newc[labels]
    return np.float32(np.mean((f64 - g) ** 2))


def kernel(batch_feature, batch_label, center_feature):
    global _LAST_RESULT
    f = np.ascontiguousarray(np.asarray(batch_feature, dtype=np.float32))
    labels = np.asarray(batch_label).astype(np.int64)
    cf = np.ascontiguousarray(np.asarray(center_feature, dtype=np.float32))

    order = np.argsort(labels, kind="stable")
    sl = labels[order]                       # sorted labels
    uniq_all, run_start, run_cnt = np.unique(sl, return_index=True,
                                             return_counts=True)

    n_pairs_total = int(((run_cnt * (run_cnt - 1)) // 2).sum())
    if n_pairs_total > HOST_PAIR_LIMIT:
        return _host_reference(f, labels, cf)

    in_maps = []
    ident = np.eye(128, dtype=np.float32)
    host_corr = 0.0                          # O(duplicates) terms, float64
    f64 = f.astype(np.float64)
    cf64 = cf.astype(np.float64)
    for k in range(NCORES):
        seg = slice(k * T, (k + 1) * T)
        rows = order[seg]
        sl_k = sl[seg]
        uniq, first_idx, cnt = np.unique(sl_k, return_index=True,
                                         return_counts=True)
        U_k = uniq.shape[0]

        tab_k = np.zeros((T, F), np.float32)
        tab_k[:U_k] = cf[uniq]
        f_k = np.zeros((T, F), np.float32)
        f_k[:U_k] = f[rows[first_idx]]

        in_maps.append({
            "ident": ident,
            "trm": np.ascontiguousarray(tab_k.reshape(128, NT * F)).astype(FP8),
            "frm": np.ascontiguousarray(f_k.reshape(128, NT * F)).astype(FP8),
        })

        dupm = cnt >= 2
        if dupm.any():
            nd = cnt[dupm].astype(np.float64)
            wq_d = 1.0 - _QCOEF / nd
            cd = cf64[uniq[dupm]]
            fd = f64[rows[first_idx[dupm]]]
            # (A) first-sample norm weight correction (w' - d^2)
            host_corr += float((wq_d - _D2) @ (fd * fd).sum(1))
            # (C) extras' center norms: d^2 (n-1) ||c||^2
            host_corr += _D2 * float((nd - 1.0) @ (cd * cd).sum(1))
            # extras: non-first samples of duplicated classes
            is_first = np.zeros(T, bool)
            is_first[first_idx] = True
            ex = rows[~is_first]
            ex_lab = labels[ex]
            fe = f64[ex]
            ce = cf64[ex_lab]
            wq_e = 1.0 - _QCOEF / cnt[np.searchsorted(uniq, ex_lab)]
            # (B) extras' feature norms, (D) extras' cross terms
            host_corr += float(wq_e @ (fe * fe).sum(1))
            host_corr -= 2.0 * _D2 * float((fe * ce).sum())

    _ensure_ntff_hook()
    from concourse.bass_utils import run_bass_kernel_spmd

    nc = _get_nc()
    res = run_bass_kernel_spmd(nc, in_maps, core_ids=list(range(NCORES)))
    _LAST_RESULT = res

    d_total = 0.0
    for r in res.results:
        d_total += float(np.asarray(r["out"], np.float64).sum())

    # same-class pair term, float64 on host (~B^2/2C pairs)
    q2 = 0.0
    dup = np.nonzero(run_cnt >= 2)[0]
    if dup.size:
        ia_l, jb_l, wt_l = [], [], []
        for r_i in dup:
            s0, n = int(run_start[r_i]), int(run_cnt[r_i])
            g = order[s0:s0 + n]
            iu, ju = np.triu_indices(n, k=1)
            ia_l.append(g[iu]); jb_l.append(g[ju])
            wt_l.append(np.full(iu.shape[0], 2.0 / n))
        ia = np.concatenate(ia_l); jb = np.concatenate(jb_l)
        wt = np.concatenate(wt_l)
        dots = np.einsum("ij,ij->i", f64[ia], f64[jb])
        q2 = float(wt @ dots)

    loss = (_D2 * d_total + host_corr - _QCOEF * q2) / (B * F)
    return np.float32(loss)
